# revision 3
# baseline (speedup 1.0000x reference)
"""Trainium2 Bass kernel for nn_BasicCSRNN (bottom-up tree RNN).

Strategy: shard H=256 across 8 cores (32 cols each) -> zero cross-core
communication. Per level, the scatter-add to parents becomes ~280 small
TensorEngine matmuls with host-built 0/1 fp16 selection matrices:
sources are sorted by parent slot (layout chosen top-down so every level
uses one consistent layout), each 128-source block hits one 64-slot dst
window. Childless/REL nodes receive zero PSUM contribution and the +cb
bias makes their hidden state tanh(cb) automatically.
"""
import os
import sys

sys.path.insert(0, "/opt/trn_rl_repo")
import numpy as np

D, W = 16, 16384
N = 1 + (D - 1) * W
H, I, E = 256, 256, 16
NCORES = 8
HS = H // NCORES  # 32
WS = 64           # dst window size (slots)
PAD_POS = -1      # parentpos for dropped/padding slots (never selected)

_cache = {}
LAST_EXEC_NS = None


def _install_profhook():
    """Register the NTFF profile hook so trace=True works under axon."""
    import types
    try:
        from antenv import axon_hooks  # noqa: F401
        return
    except ImportError:
        pass
    import antenv
    mod = types.ModuleType("antenv.axon_hooks")
    _hook = [None]
    mod.set_axon_ntff_profile_hook = lambda h: _hook.__setitem__(0, h)
    mod.get_axon_ntff_profile_hook = lambda: _hook[0]
    sys.modules["antenv.axon_hooks"] = mod
    antenv.axon_hooks = mod
    from trn_agent_boot.trn_boot import _ntff_profile_via_ctypes
    mod.set_axon_ntff_profile_hook(
        _ntff_profile_via_ctypes("/opt/axon/libaxon_pjrt.so"))
    import concourse.bass_utils as bu
    bu.upload_artifacts = lambda tmpdir: "local://" + str(tmpdir)


def _build_structure(parent, levels, is_rel):
    """Host-side layout build. Returns per-level slot layouts, window/entry
    lists and the fp16 selection-matrix streams (core-independent)."""
    lv = [np.asarray(levels[d], np.int64) for d in range(D - 1)]
    # children counts per node (from each level's parents)
    cnt = np.zeros(N, np.int64)
    for d in range(D - 1):
        np.add.at(cnt, parent[lv[d]], 1)

    slotpos = np.full(N, -1, np.int64)   # node -> slot within its level
    layouts = []  # per level d: dict(slot_node: [NSLOT] node id or -1)

    # ---- level 0 layout: arbitrary order, pad to 128 ----
    nodes0 = lv[0]
    nslot0 = ((len(nodes0) + 127) // 128) * 128
    slot_node0 = np.full(nslot0, -1, np.int64)
    slot_node0[:len(nodes0)] = nodes0
    slotpos[nodes0] = np.arange(len(nodes0))
    layouts.append({"slot_node": slot_node0, "nslot": nslot0})

    # ---- levels 1..14: group by parent window, bin-pack into 128-blocks ----
    for d in range(1, D - 1):
        nodes = lv[d]
        par = parent[nodes]
        dropped = is_rel[par]            # children of REL parents: no sel row
        ppos = slotpos[par]
        win = ppos // WS                 # parent window id
        # order: non-dropped sorted by window, then dropped (fake window = big)
        wkey = np.where(dropped, 1 << 40, win)
        order = np.argsort(wkey, kind="stable")
        snodes = nodes[order]
        swin = wkey[order]
        # bin-pack: consecutive whole windows into 128-slot blocks; a window's
        # nodes must not straddle a block (unless window itself > 128).
        slot_node = []
        i = 0
        nnodes = len(snodes)
        cur = 0  # slots used in current block
        while i < nnodes:
            j = i + 1
            while j < nnodes and swin[j] == swin[i]:
                j += 1
            g = j - i  # nodes in this window
            if g > 128 - cur and cur > 0:
                slot_node += [-1] * (128 - cur)   # pad out block
                cur = 0
            take = i
            while g > 128:
                # oversized window: split across dedicated blocks
                slot_node += list(snodes[take:take + 128])
                take += 128
                g -= 128
                cur = 0
            slot_node += list(snodes[take:j])
            cur = (cur + g) % 128
            i = j
        if len(slot_node) % 128:
            slot_node += [-1] * (128 - len(slot_node) % 128)
        slot_node = np.array(slot_node, np.int64)
        nslot = len(slot_node)
        real = slot_node >= 0
        slotpos[slot_node[real]] = np.nonzero(real)[0]
        layouts.append({"slot_node": slot_node, "nslot": nslot})

    # ---- per-transition (d -> d-1) entries and sel streams ----
    transitions = []  # d=1..14: dict(entries, sel, nb, nw_prev)
    for d in range(1, D - 1):
        L = layouts[d]
        slot_node = L["slot_node"]
        nb = L["nslot"] // 128
        nw_prev = layouts[d - 1]["nslot"] // WS
        # per slot: parentpos (or PAD)
        sp = np.full(L["nslot"], PAD_POS, np.int64)
        real = slot_node >= 0
        rn = slot_node[real]
        keep = ~is_rel[parent[rn]]
        idx = np.nonzero(real)[0][keep]
        sp[idx] = slotpos[parent[slot_node[idx]]]
        # entries: (block s, window t) for windows present in block
        entries = []  # list of (s, t)
        win_of = np.where(sp >= 0, sp // WS, -1).reshape(nb, 128)
        for s in range(nb):
            ws_here = np.unique(win_of[s])
            for t in ws_here:
                if t >= 0:
                    entries.append((s, int(t)))
        # every window needs at least one entry (else PSUM slice is never
        # written); empty windows get a dummy all-zero sel entry
        have = {t for _, t in entries}
        for t in range(nw_prev):
            if t not in have:
                entries.append((-1, t))
        # order entries by (t, s) so same-window entries are adjacent
        entries.sort(key=lambda e: (e[1], e[0]))
        ne = len(entries)
        sel = np.zeros((128, ne * WS), np.float16)
        spb = sp.reshape(nb, 128)
        for e, (s, t) in enumerate(entries):
            if s < 0:
                continue
            rows = spb[s]
            k = np.nonzero((rows >= t * WS) & (rows < (t + 1) * WS))[0]
            sel[k, e * WS + (rows[k] - t * WS)] = 1.0
        transitions.append({"entries": entries, "sel": sel, "nb": nb,
                            "nw_prev": nw_prev, "d": d})
    return layouts, transitions


def _compile(layouts, transitions):
    import concourse.bass as bass
    import concourse.bacc as bacc
    import concourse.mybir as mybir
    import concourse.tile as tile

    f32 = mybir.dt.float32
    f16 = mybir.dt.float16

    nc = bacc.Bacc("TRN2", target_bir_lowering=False, debug=False,
                   num_devices=NCORES)
    nb = [L["nslot"] // 128 for L in layouts]
    m14_in = nc.dram_tensor("m14", [128, nb[14] * HS], f16, kind="ExternalInput")
    cb_in = nc.dram_tensor("cb", [128, 512], f32, kind="ExternalInput")
    sel_in = {}
    for tr in transitions:
        d = tr["d"]
        ne = len(tr["entries"])
        sel_in[d] = nc.dram_tensor(f"sel{d}", [128, ne * WS], f16,
                                   kind="ExternalInput")
    scale_in = {d: nc.dram_tensor(f"scale{d}", [128, nb[d] * HS], f16,
                                  kind="ExternalInput")
                for d in range(D - 2)}  # levels 0..13 (level 14 folded in m14)
    ones_in = nc.dram_tensor("ones", [128, 1], f32, kind="ExternalInput")
    root_out = nc.dram_tensor("root", [1, HS], f32, kind="ExternalOutput")

    SELCH = 48  # sel entries per DMA chunk

    with tile.TileContext(nc) as tc:
        with tc.tile_pool(name="const", bufs=1) as cpool, \
             tc.tile_pool(name="mwork", bufs=2) as mpool, \
             tc.tile_pool(name="work", bufs=2) as pool, \
             tc.tile_pool(name="selp", bufs=3) as selpool, \
             tc.tile_pool(name="psum", bufs=4, space="PSUM") as psum_pool:
            cb_t = cpool.tile([128, 512], f32, tag="cb")
            nc.sync.dma_start(out=cb_t[:], in_=cb_in[:])
            ones_t = cpool.tile([128, 1], f32, tag="ones")
            nc.sync.dma_start(out=ones_t[:], in_=ones_in[:])

            m_t = mpool.tile([128, nb[14] * HS], f16, tag="m")
            nc.sync.dma_start(out=m_t[:], in_=m14_in[:])

            for tr in reversed(transitions):   # d = 14 .. 1
                d = tr["d"]
                dd = d - 1                      # destination level
                entries = tr["entries"]
                ne = len(entries)
                nw = tr["nw_prev"]
                # load sel chunks
                sel_tiles = []
                for c in range(0, ne, SELCH):
                    hi = min(c + SELCH, ne)
                    st = selpool.tile([128, SELCH * WS], f16, tag="sel")
                    nc.sync.dma_start(out=st[:, :(hi - c) * WS],
                                      in_=sel_in[d][:, c * WS:hi * WS])
                    sel_tiles.append(st)
                sc_t = pool.tile([128, nb[dd] * HS], f16, tag="scale")
                nc.sync.dma_start(out=sc_t[:], in_=scale_in[dd][:])

                hpre_t = pool.tile([128, nb[dd] * HS], f32, tag="hpre")
                # group windows by psum tile (32 windows of 64 = 16 blocks)
                NG = (nw + 31) // 32
                eidx = 0
                for g in range(NG):
                    wlo, whi = g * 32, min((g + 1) * 32, nw)
                    ps = psum_pool.tile([128, 512], f32, tag="ps")
                    covered = set()
                    while eidx < ne and entries[eidx][1] < whi:
                        s, t = entries[eidx]
                        first = t not in covered
                        covered.add(t)
                        last = (eidx + 1 == ne) or (entries[eidx + 1][1] != t)
                        half = (t % 2) * 64
                        sl = (t // 2) % 16
                        st = sel_tiles[eidx // SELCH]
                        off = (eidx % SELCH) * WS
                        nc.tensor.matmul(
                            out=ps[half:half + 64, sl * HS:(sl + 1) * HS],
                            lhsT=st[:, off:off + WS],
                            rhs=m_t[:, max(s, 0) * HS:(max(s, 0) + 1) * HS],
                            start=first, stop=last)
                        eidx += 1
                    npair = (whi - wlo) // 2
                    nc.vector.tensor_tensor(
                        out=hpre_t[:, g * 512:g * 512 + npair * HS],
                        in0=ps[:, :npair * HS], in1=cb_t[:, :npair * HS],
                        op=mybir.AluOpType.add)
                h_t = pool.tile([128, nb[dd] * HS], f32, tag="h")
                nc.scalar.activation(out=h_t[:], in_=hpre_t[:],
                                     func=mybir.ActivationFunctionType.Tanh)
                m_t = mpool.tile([128, nb[dd] * HS], f16, tag="m")
                nc.vector.tensor_tensor(out=m_t[:], in0=h_t[:], in1=sc_t[:],
                                        op=mybir.AluOpType.mult)

            # ---- root reduce: sum all m_0 rows ----
            red_t = pool.tile([128, HS], f32, tag="red")
            m0 = m_t[:]
            nc.vector.tensor_reduce(
                out=red_t[:],
                in_=bass.AP(m0.tensor, m0.offset,
                            [[nb[0] * HS, 128], [1, HS], [HS, nb[0]]]),
                axis=mybir.AxisListType.X,
                op=mybir.AluOpType.add)
            rps = psum_pool.tile([128, HS], f32, tag="rp")
            nc.tensor.matmul(out=rps[0:1, :], lhsT=ones_t[:], rhs=red_t[:],
                             start=True, stop=True)
            rout = pool.tile([1, HS], f32, tag="ro")
            nc.vector.tensor_copy(out=rout[:], in_=rps[0:1, :])
            nc.sync.dma_start(out=root_out[:], in_=rout[:])

    nc.finalize()
    return nc


def kernel(embedding, Wx, We, b, parent, etype, levels, is_rel):
    from concourse.bass_utils import run_bass_kernel_spmd

    embedding = np.asarray(embedding, np.float32)
    Wx = np.asarray(Wx, np.float32)
    We = np.asarray(We, np.float32)
    b = np.asarray(b, np.float32)
    parent = np.asarray(parent, np.int64)
    etype = np.asarray(etype, np.int64)
    levels_np = np.asarray(levels, np.int64)
    is_rel = np.asarray(is_rel, bool)

    key = (parent.tobytes(), is_rel.tobytes(), levels_np.tobytes())
    import hashlib
    key = hashlib.sha1(b"".join(key)).hexdigest()
    if key not in _cache:
        layouts, transitions = _build_structure(parent, levels_np, is_rel)
        nc = _compile(layouts, transitions)
        _cache[key] = (layouts, transitions, nc)
    layouts, transitions, nc = _cache[key]

    # ---- numeric inputs ----
    c = embedding @ Wx                       # [H]
    cb_full = c + b[0]                       # [H]
    tanhcb = np.tanh(cb_full)
    WeT = We[:, 0, :]                        # [E, H]

    nbs = [L["nslot"] // 128 for L in layouts]
    in_maps = []
    for core in range(NCORES):
        cs = slice(core * HS, (core + 1) * HS)
        cb_c = cb_full[cs]
        m = {"cb": np.tile(cb_c, (128, 16)).astype(np.float32),
             "ones": np.ones((128, 1), np.float32)}
        for tr in transitions:
            m[f"sel{tr['d']}"] = tr["sel"]
        # scale per level 0..13; m14 for level 14
        for d in range(D - 1):
            L = layouts[d]
            sn = L["slot_node"]
            nb = nbs[d]
            sc = np.zeros((L["nslot"], HS), np.float32)
            real = sn >= 0
            sc[real] = WeT[etype[sn[real]]][:, cs]
            scr = sc.reshape(nb, 128, HS).transpose(1, 0, 2).reshape(128, nb * HS)
            if d == D - 2:
                m["m14"] = (scr * np.tile(tanhcb[cs], nb)[None, :]).astype(np.float16)
            else:
                m[f"scale{d}"] = scr.astype(np.float16)
        in_maps.append(m)

    trace = bool(os.environ.get("CSRNN_TRACE"))
    kw = {}
    if trace:
        import tempfile
        _install_profhook()
        kw = {"trace": True, "tmpdir": tempfile.mkdtemp(prefix="csrnn_")}
    res = run_bass_kernel_spmd(nc, in_maps, list(range(NCORES)), **kw)
    global LAST_EXEC_NS
    LAST_EXEC_NS = res.exec_time_ns
    acc0 = np.concatenate([res.results[core]["root"][0] for core in range(NCORES)])
    root_hidden = np.zeros(H, np.float32) if is_rel[0] else acc0
    out = np.tanh(c + root_hidden + b[0])
    return out[None, :].astype(np.float32)


# revision 4
# speedup vs baseline: 1.1828x; 1.1828x over previous
"""Trainium2 Bass kernel for nn_BasicCSRNN (bottom-up tree RNN).

Strategy: shard H=256 across 8 cores (32 cols each) -> zero cross-core
communication. Per level, the scatter-add to parents becomes ~280 small
TensorEngine matmuls with host-built 0/1 fp16 selection matrices:
sources are sorted by parent slot (layout chosen top-down so every level
uses one consistent layout), each 128-source block hits one 64-slot dst
window. Childless/REL nodes receive zero PSUM contribution and the +cb
bias makes their hidden state tanh(cb) automatically.
"""
import os
import sys

sys.path.insert(0, "/opt/trn_rl_repo")
import numpy as np

D, W = 16, 16384
N = 1 + (D - 1) * W
H, I, E = 256, 256, 16
NCORES = 8
HS = H // NCORES  # 32
WS = 128          # dst window size (slots) = one 128-slot block
PAD_POS = -1      # parentpos for dropped/padding slots (never selected)

_cache = {}
LAST_EXEC_NS = None


def _install_profhook():
    """Register the NTFF profile hook so trace=True works under axon."""
    import types
    try:
        from antenv import axon_hooks  # noqa: F401
        return
    except ImportError:
        pass
    import antenv
    mod = types.ModuleType("antenv.axon_hooks")
    _hook = [None]
    mod.set_axon_ntff_profile_hook = lambda h: _hook.__setitem__(0, h)
    mod.get_axon_ntff_profile_hook = lambda: _hook[0]
    sys.modules["antenv.axon_hooks"] = mod
    antenv.axon_hooks = mod
    from trn_agent_boot.trn_boot import _ntff_profile_via_ctypes
    mod.set_axon_ntff_profile_hook(
        _ntff_profile_via_ctypes("/opt/axon/libaxon_pjrt.so"))
    import concourse.bass_utils as bu
    bu.upload_artifacts = lambda tmpdir: "local://" + str(tmpdir)


def _build_structure(parent, levels, is_rel):
    """Host-side layout build. Returns per-level slot layouts, window/entry
    lists and the fp16 selection-matrix streams (core-independent)."""
    lv = [np.asarray(levels[d], np.int64) for d in range(D - 1)]
    # children counts per node (from each level's parents)
    cnt = np.zeros(N, np.int64)
    for d in range(D - 1):
        np.add.at(cnt, parent[lv[d]], 1)

    slotpos = np.full(N, -1, np.int64)   # node -> slot within its level
    layouts = []  # per level d: dict(slot_node: [NSLOT] node id or -1)

    # ---- level 0 layout: arbitrary order, pad to 128 ----
    nodes0 = lv[0]
    nslot0 = ((len(nodes0) + 127) // 128) * 128
    slot_node0 = np.full(nslot0, -1, np.int64)
    slot_node0[:len(nodes0)] = nodes0
    slotpos[nodes0] = np.arange(len(nodes0))
    layouts.append({"slot_node": slot_node0, "nslot": nslot0})

    # ---- levels 1..14: group by parent window, bin-pack into 128-blocks ----
    for d in range(1, D - 1):
        nodes = lv[d]
        par = parent[nodes]
        dropped = is_rel[par]            # children of REL parents: no sel row
        ppos = slotpos[par]
        win = ppos // WS                 # parent window id
        # order: non-dropped sorted by window, then dropped (fake window = big)
        wkey = np.where(dropped, 1 << 40, win)
        order = np.argsort(wkey, kind="stable")
        snodes = nodes[order]
        swin = wkey[order]
        # bin-pack: consecutive whole windows into 128-slot blocks; a window's
        # nodes must not straddle a block (unless window itself > 128).
        slot_node = []
        i = 0
        nnodes = len(snodes)
        cur = 0  # slots used in current block
        while i < nnodes:
            j = i + 1
            while j < nnodes and swin[j] == swin[i]:
                j += 1
            g = j - i  # nodes in this window
            if g > 128 - cur and cur > 0:
                slot_node += [-1] * (128 - cur)   # pad out block
                cur = 0
            take = i
            while g > 128:
                # oversized window: split across dedicated blocks
                slot_node += list(snodes[take:take + 128])
                take += 128
                g -= 128
                cur = 0
            slot_node += list(snodes[take:j])
            cur = (cur + g) % 128
            i = j
        if len(slot_node) % 128:
            slot_node += [-1] * (128 - len(slot_node) % 128)
        slot_node = np.array(slot_node, np.int64)
        nslot = len(slot_node)
        real = slot_node >= 0
        slotpos[slot_node[real]] = np.nonzero(real)[0]
        layouts.append({"slot_node": slot_node, "nslot": nslot})

    # ---- per-transition (d -> d-1) entries and sel streams ----
    transitions = []  # d=1..14: dict(entries, sel, nb, nw_prev)
    for d in range(1, D - 1):
        L = layouts[d]
        slot_node = L["slot_node"]
        nb = L["nslot"] // 128
        nw_prev = layouts[d - 1]["nslot"] // WS
        # per slot: parentpos (or PAD)
        sp = np.full(L["nslot"], PAD_POS, np.int64)
        real = slot_node >= 0
        rn = slot_node[real]
        keep = ~is_rel[parent[rn]]
        idx = np.nonzero(real)[0][keep]
        sp[idx] = slotpos[parent[slot_node[idx]]]
        # entries: (block s, window t) for windows present in block
        entries = []  # list of (s, t)
        win_of = np.where(sp >= 0, sp // WS, -1).reshape(nb, 128)
        for s in range(nb):
            ws_here = np.unique(win_of[s])
            for t in ws_here:
                if t >= 0:
                    entries.append((s, int(t)))
        # every window needs at least one entry (else PSUM slice is never
        # written); empty windows get a dummy all-zero sel entry
        have = {t for _, t in entries}
        for t in range(nw_prev):
            if t not in have:
                entries.append((-1, t))
        # order entries by (t, s) so same-window entries are adjacent
        entries.sort(key=lambda e: (e[1], e[0]))
        ne = len(entries)
        sel = np.zeros((128, ne * WS), np.float16)
        spb = sp.reshape(nb, 128)
        for e, (s, t) in enumerate(entries):
            if s < 0:
                continue
            rows = spb[s]
            k = np.nonzero((rows >= t * WS) & (rows < (t + 1) * WS))[0]
            sel[k, e * WS + (rows[k] - t * WS)] = 1.0
        transitions.append({"entries": entries, "sel": sel, "nb": nb,
                            "nw_prev": nw_prev, "d": d})
    return layouts, transitions


def _compile(layouts, transitions):
    import concourse.bass as bass
    import concourse.bacc as bacc
    import concourse.mybir as mybir
    import concourse.tile as tile

    f32 = mybir.dt.float32
    f16 = mybir.dt.float16

    nc = bacc.Bacc("TRN2", target_bir_lowering=False, debug=False,
                   num_devices=NCORES)
    nb = [L["nslot"] // 128 for L in layouts]
    ng = [(b + 15) // 16 for b in nb]
    m14_in = nc.dram_tensor("m14", [128, nb[14] * HS], f16, kind="ExternalInput")
    cb_in = nc.dram_tensor("cb", [128, 512], f32, kind="ExternalInput")
    sel_in = {}
    for tr in transitions:
        d = tr["d"]
        ne = len(tr["entries"])
        sel_in[d] = nc.dram_tensor(f"sel{d}", [128, ne * WS], f16,
                                   kind="ExternalInput")
    scale_in = {d: nc.dram_tensor(f"scale{d}", [128, nb[d] * HS], f16,
                                  kind="ExternalInput")
                for d in range(D - 2)}  # levels 0..13 (level 14 folded in m14)
    ones_in = nc.dram_tensor("ones", [128, 1], f32, kind="ExternalInput")
    root_out = nc.dram_tensor("root", [1, HS], f32, kind="ExternalOutput")

    SELCH = 16  # sel entries per DMA chunk

    with tile.TileContext(nc) as tc:
        with tc.tile_pool(name="const", bufs=1) as cpool, \
             tc.tile_pool(name="work", bufs=2) as pool, \
             tc.tile_pool(name="selp", bufs=3) as selpool, \
             tc.tile_pool(name="psum", bufs=4, space="PSUM") as psum_pool:
            cb_t = cpool.tile([128, 512], f32, tag="cb")
            nc.sync.dma_start(out=cb_t[:], in_=cb_in[:])
            ones_t = cpool.tile([128, 1], f32, tag="ones")
            nc.sync.dma_start(out=ones_t[:], in_=ones_in[:])

            m14_t = cpool.tile([128, nb[14] * HS], f16, tag="m14")
            nc.sync.dma_start(out=m14_t[:], in_=m14_in[:])
            # m chunks: list of (tile, elem offset) per 16-block group
            m_chunks = [(m14_t, 512 * g) for g in range(ng[14])]

            for tr in reversed(transitions):   # d = 14 .. 1
                d = tr["d"]
                dd = d - 1                      # destination level
                entries = tr["entries"]
                ne = len(entries)
                nwd = nb[dd]                    # windows = blocks of dest level
                sel_tiles = []
                for c in range(0, ne, SELCH):
                    hi = min(c + SELCH, ne)
                    st = selpool.tile([128, SELCH * WS], f16, tag="sel")
                    nc.sync.dma_start(out=st[:, :(hi - c) * WS],
                                      in_=sel_in[d][:, c * WS:hi * WS])
                    sel_tiles.append(st)
                sc_t = pool.tile([128, nb[dd] * HS], f16, tag="scale")
                nc.sync.dma_start(out=sc_t[:], in_=scale_in[dd][:])

                new_chunks = []
                eidx = 0
                for g in range(ng[dd]):
                    wlo, whi = g * 16, min((g + 1) * 16, nwd)
                    nwin = whi - wlo
                    ps = psum_pool.tile([128, 512], f32, tag="ps")
                    covered = set()
                    while eidx < ne and entries[eidx][1] < whi:
                        s, t = entries[eidx]
                        first = t not in covered
                        covered.add(t)
                        last = (eidx + 1 == ne) or (entries[eidx + 1][1] != t)
                        st = sel_tiles[eidx // SELCH]
                        off = (eidx % SELCH) * WS
                        su = max(s, 0)
                        mt, mo = m_chunks[su // 16]
                        nc.tensor.matmul(
                            out=ps[:, (t % 16) * HS:(t % 16 + 1) * HS],
                            lhsT=st[:, off:off + WS],
                            rhs=mt[:, mo + (su % 16) * HS:mo + (su % 16 + 1) * HS],
                            start=first, stop=last)
                        eidx += 1
                    hp = pool.tile([128, nwin * HS], f32, tag=f"hp{g % 4}")
                    nc.vector.tensor_tensor(
                        out=hp[:], in0=ps[:, :nwin * HS],
                        in1=cb_t[:, :nwin * HS], op=mybir.AluOpType.add)
                    hh = pool.tile([128, nwin * HS], f16, tag=f"hh{g % 4}")
                    nc.scalar.activation(out=hh[:], in_=hp[:],
                                         func=mybir.ActivationFunctionType.Tanh)
                    mc = pool.tile([128, nwin * HS], f16, tag=f"mc{g % 8}")
                    nc.vector.tensor_tensor(
                        out=mc[:], in0=hh[:],
                        in1=sc_t[:, g * 512:g * 512 + nwin * HS],
                        op=mybir.AluOpType.mult)
                    new_chunks.append((mc, 0))
                m_chunks = new_chunks

            # ---- root reduce: sum all m_0 rows ----
            red_t = pool.tile([128, HS], f32, tag="red")
            for g, (mt, mo) in enumerate(m_chunks):
                nblk = min(16, nb[0] - g * 16)
                rc = pool.tile([128, HS], f32, tag="redc")
                ap = mt[:]
                nc.vector.tensor_reduce(
                    out=rc[:],
                    in_=bass.AP(ap.tensor, ap.offset + mo,
                                [[ap.ap[0][0], 128], [1, HS], [HS, nblk]]),
                    axis=mybir.AxisListType.X,
                    op=mybir.AluOpType.add)
                if g == 0:
                    nc.vector.tensor_copy(out=red_t[:], in_=rc[:])
                else:
                    nc.vector.tensor_tensor(out=red_t[:], in0=red_t[:],
                                            in1=rc[:], op=mybir.AluOpType.add)
            rps = psum_pool.tile([128, HS], f32, tag="rp")
            nc.tensor.matmul(out=rps[0:1, :], lhsT=ones_t[:], rhs=red_t[:],
                             start=True, stop=True)
            rout = pool.tile([1, HS], f32, tag="ro")
            nc.vector.tensor_copy(out=rout[:], in_=rps[0:1, :])
            nc.sync.dma_start(out=root_out[:], in_=rout[:])

    nc.finalize()
    return nc


def kernel(embedding, Wx, We, b, parent, etype, levels, is_rel):
    from concourse.bass_utils import run_bass_kernel_spmd

    embedding = np.asarray(embedding, np.float32)
    Wx = np.asarray(Wx, np.float32)
    We = np.asarray(We, np.float32)
    b = np.asarray(b, np.float32)
    parent = np.asarray(parent, np.int64)
    etype = np.asarray(etype, np.int64)
    levels_np = np.asarray(levels, np.int64)
    is_rel = np.asarray(is_rel, bool)

    key = (parent.tobytes(), is_rel.tobytes(), levels_np.tobytes())
    import hashlib
    key = hashlib.sha1(b"".join(key)).hexdigest()
    if key not in _cache:
        layouts, transitions = _build_structure(parent, levels_np, is_rel)
        nc = _compile(layouts, transitions)
        _cache[key] = (layouts, transitions, nc)
    layouts, transitions, nc = _cache[key]

    # ---- numeric inputs ----
    c = embedding @ Wx                       # [H]
    cb_full = c + b[0]                       # [H]
    tanhcb = np.tanh(cb_full)
    WeT = We[:, 0, :]                        # [E, H]

    nbs = [L["nslot"] // 128 for L in layouts]
    in_maps = []
    for core in range(NCORES):
        cs = slice(core * HS, (core + 1) * HS)
        cb_c = cb_full[cs]
        m = {"cb": np.tile(cb_c, (128, 16)).astype(np.float32),
             "ones": np.ones((128, 1), np.float32)}
        for tr in transitions:
            m[f"sel{tr['d']}"] = tr["sel"]
        # scale per level 0..13; m14 for level 14
        for d in range(D - 1):
            L = layouts[d]
            sn = L["slot_node"]
            nb = nbs[d]
            sc = np.zeros((L["nslot"], HS), np.float32)
            real = sn >= 0
            sc[real] = WeT[etype[sn[real]]][:, cs]
            scr = sc.reshape(nb, 128, HS).transpose(1, 0, 2).reshape(128, nb * HS)
            if d == D - 2:
                m["m14"] = (scr * np.tile(tanhcb[cs], nb)[None, :]).astype(np.float16)
            else:
                m[f"scale{d}"] = scr.astype(np.float16)
        in_maps.append(m)

    trace = bool(os.environ.get("CSRNN_TRACE"))
    kw = {}
    if trace:
        import tempfile
        _install_profhook()
        kw = {"trace": True, "tmpdir": tempfile.mkdtemp(prefix="csrnn_")}
    res = run_bass_kernel_spmd(nc, in_maps, list(range(NCORES)), **kw)
    global LAST_EXEC_NS
    LAST_EXEC_NS = res.exec_time_ns
    acc0 = np.concatenate([res.results[core]["root"][0] for core in range(NCORES)])
    root_hidden = np.zeros(H, np.float32) if is_rel[0] else acc0
    out = np.tanh(c + root_hidden + b[0])
    return out[None, :].astype(np.float32)


# revision 5
# speedup vs baseline: 1.6647x; 1.4074x over previous
"""Trainium2 Bass kernel for nn_BasicCSRNN (bottom-up tree RNN).

Strategy: shard H=256 across 8 cores (32 cols each) -> zero cross-core
communication. Per level, the scatter-add to parents becomes ~280 small
TensorEngine matmuls with host-built 0/1 fp16 selection matrices:
sources are sorted by parent slot (layout chosen top-down so every level
uses one consistent layout), each 128-source block hits one 64-slot dst
window. Childless/REL nodes receive zero PSUM contribution and the +cb
bias makes their hidden state tanh(cb) automatically.
"""
import os
import sys

sys.path.insert(0, "/opt/trn_rl_repo")
import numpy as np

D, W = 16, 16384
N = 1 + (D - 1) * W
H, I, E = 256, 256, 16
NCORES = 8
HS = H // NCORES  # 32
WS = 128          # dst window size (slots) = one 128-slot block
PAD_POS = -1      # parentpos for dropped/padding slots (never selected)

_cache = {}
LAST_EXEC_NS = None


def _install_profhook():
    """Register the NTFF profile hook so trace=True works under axon."""
    import types
    try:
        from antenv import axon_hooks  # noqa: F401
        return
    except ImportError:
        pass
    import antenv
    mod = types.ModuleType("antenv.axon_hooks")
    _hook = [None]
    mod.set_axon_ntff_profile_hook = lambda h: _hook.__setitem__(0, h)
    mod.get_axon_ntff_profile_hook = lambda: _hook[0]
    sys.modules["antenv.axon_hooks"] = mod
    antenv.axon_hooks = mod
    from trn_agent_boot.trn_boot import _ntff_profile_via_ctypes
    mod.set_axon_ntff_profile_hook(
        _ntff_profile_via_ctypes("/opt/axon/libaxon_pjrt.so"))
    import concourse.bass_utils as bu
    bu.upload_artifacts = lambda tmpdir: "local://" + str(tmpdir)


def _build_structure(parent, levels, is_rel):
    """Host-side layout build. Returns per-level slot layouts, window/entry
    lists and the fp16 selection-matrix streams (core-independent)."""
    lv = [np.asarray(levels[d], np.int64) for d in range(D - 1)]
    # children counts per node (from each level's parents)
    cnt = np.zeros(N, np.int64)
    for d in range(D - 1):
        np.add.at(cnt, parent[lv[d]], 1)

    slotpos = np.full(N, -1, np.int64)   # node -> slot within its level
    layouts = []  # per level d: dict(slot_node: [NSLOT] node id or -1)

    # ---- level 0 layout: arbitrary order, pad to 128 ----
    nodes0 = lv[0]
    nslot0 = ((len(nodes0) + 127) // 128) * 128
    slot_node0 = np.full(nslot0, -1, np.int64)
    slot_node0[:len(nodes0)] = nodes0
    slotpos[nodes0] = np.arange(len(nodes0))
    layouts.append({"slot_node": slot_node0, "nslot": nslot0})

    # ---- levels 1..14: group by parent window, bin-pack into 128-blocks ----
    for d in range(1, D - 1):
        nodes = lv[d]
        par = parent[nodes]
        dropped = is_rel[par]            # children of REL parents: no sel row
        ppos = slotpos[par]
        win = ppos // WS                 # parent window id
        # order: non-dropped sorted by window, then dropped (fake window = big)
        wkey = np.where(dropped, 1 << 40, win)
        order = np.argsort(wkey, kind="stable")
        snodes = nodes[order]
        swin = wkey[order]
        # bin-pack: consecutive whole windows into 128-slot blocks; a window's
        # nodes must not straddle a block (unless window itself > 128).
        slot_node = []
        i = 0
        nnodes = len(snodes)
        cur = 0  # slots used in current block
        while i < nnodes:
            j = i + 1
            while j < nnodes and swin[j] == swin[i]:
                j += 1
            g = j - i  # nodes in this window
            if g > 128 - cur and cur > 0:
                slot_node += [-1] * (128 - cur)   # pad out block
                cur = 0
            take = i
            while g > 128:
                # oversized window: split across dedicated blocks
                slot_node += list(snodes[take:take + 128])
                take += 128
                g -= 128
                cur = 0
            slot_node += list(snodes[take:j])
            cur = (cur + g) % 128
            i = j
        if len(slot_node) % 128:
            slot_node += [-1] * (128 - len(slot_node) % 128)
        slot_node = np.array(slot_node, np.int64)
        nslot = len(slot_node)
        real = slot_node >= 0
        slotpos[slot_node[real]] = np.nonzero(real)[0]
        layouts.append({"slot_node": slot_node, "nslot": nslot})

    # ---- per-transition (d -> d-1) entries and sel streams ----
    transitions = []  # d=1..14: dict(entries, sel, nb, nw_prev)
    for d in range(1, D - 1):
        L = layouts[d]
        slot_node = L["slot_node"]
        nb = L["nslot"] // 128
        nw_prev = layouts[d - 1]["nslot"] // WS
        # per slot: parentpos (or PAD)
        sp = np.full(L["nslot"], PAD_POS, np.int64)
        real = slot_node >= 0
        rn = slot_node[real]
        keep = ~is_rel[parent[rn]]
        idx = np.nonzero(real)[0][keep]
        sp[idx] = slotpos[parent[slot_node[idx]]]
        # entries: (block s, window t) for windows present in block
        entries = []  # list of (s, t)
        win_of = np.where(sp >= 0, sp // WS, -1).reshape(nb, 128)
        for s in range(nb):
            ws_here = np.unique(win_of[s])
            for t in ws_here:
                if t >= 0:
                    entries.append((s, int(t)))
        # every window needs at least one entry (else PSUM slice is never
        # written); empty windows get a dummy all-zero sel entry
        have = {t for _, t in entries}
        for t in range(nw_prev):
            if t not in have:
                entries.append((-1, t))
        # order entries by (t, s) so same-window entries are adjacent
        entries.sort(key=lambda e: (e[1], e[0]))
        ne = len(entries)
        import ml_dtypes
        sel = np.zeros((128, ne * WS), ml_dtypes.float8_e4m3)
        spb = sp.reshape(nb, 128)
        for e, (s, t) in enumerate(entries):
            if s < 0:
                continue
            rows = spb[s]
            k = np.nonzero((rows >= t * WS) & (rows < (t + 1) * WS))[0]
            sel[k, e * WS + (rows[k] - t * WS)] = 1.0
        transitions.append({"entries": entries, "sel": sel, "nb": nb,
                            "nw_prev": nw_prev, "d": d})
    return layouts, transitions


def _compile(layouts, transitions):
    import concourse.bass as bass
    import concourse.bacc as bacc
    import concourse.mybir as mybir
    import concourse.tile as tile

    f32 = mybir.dt.float32
    f16 = mybir.dt.float16
    f8 = mybir.dt.float8e4

    nc = bacc.Bacc("TRN2", target_bir_lowering=False, debug=False,
                   num_devices=NCORES)
    nb = [L["nslot"] // 128 for L in layouts]
    ng = [(b + 15) // 16 for b in nb]
    m14_in = nc.dram_tensor("m14", [128, nb[14] * HS], f16, kind="ExternalInput")
    cb_in = nc.dram_tensor("cb", [128, 512], f32, kind="ExternalInput")
    sel_in = {}
    for tr in transitions:
        d = tr["d"]
        ne = len(tr["entries"])
        sel_in[d] = nc.dram_tensor(f"sel{d}", [128, ne * WS], f8,
                                   kind="ExternalInput")
    scale_in = {d: nc.dram_tensor(f"scale{d}", [128, nb[d] * HS], f16,
                                  kind="ExternalInput")
                for d in range(D - 2)}  # levels 0..13 (level 14 folded in m14)
    ones_in = nc.dram_tensor("ones", [128, 1], f32, kind="ExternalInput")
    root_out = nc.dram_tensor("root", [1, HS], f32, kind="ExternalOutput")

    SELCH = 16  # sel entries per DMA chunk

    with tile.TileContext(nc) as tc:
        with tc.tile_pool(name="const", bufs=1) as cpool, \
             tc.tile_pool(name="work", bufs=2) as pool, \
             tc.tile_pool(name="selp", bufs=4) as selpool, \
             tc.tile_pool(name="psum", bufs=4, space="PSUM") as psum_pool:
            cb_t = cpool.tile([128, 512], f32, tag="cb")
            nc.sync.dma_start(out=cb_t[:], in_=cb_in[:])
            ones_t = cpool.tile([128, 1], f32, tag="ones")
            nc.sync.dma_start(out=ones_t[:], in_=ones_in[:])

            m14_t = cpool.tile([128, nb[14] * HS], f16, tag="m14")
            nc.sync.dma_start(out=m14_t[:], in_=m14_in[:])
            # m chunks: list of (tile, elem offset) per 16-block group
            m_chunks = [(m14_t, 512 * g) for g in range(ng[14])]

            for tr in reversed(transitions):   # d = 14 .. 1
                d = tr["d"]
                dd = d - 1                      # destination level
                entries = tr["entries"]
                ne = len(entries)
                nwd = nb[dd]                    # windows = blocks of dest level
                sel_tiles = []
                for c in range(0, ne, SELCH):
                    hi = min(c + SELCH, ne)
                    st = selpool.tile([128, SELCH * WS], f8, tag="sel")
                    nc.sync.dma_start(out=st[:, :(hi - c) * WS],
                                      in_=sel_in[d][:, c * WS:hi * WS])
                    sel_tiles.append(st)
                sc_t = pool.tile([128, nb[dd] * HS], f16, tag="scale")
                nc.sync.dma_start(out=sc_t[:], in_=scale_in[dd][:])

                new_chunks = []
                eidx = 0
                for g in range(ng[dd]):
                    wlo, whi = g * 16, min((g + 1) * 16, nwd)
                    nwin = whi - wlo
                    ps = psum_pool.tile([128, 512], f32, tag="ps")
                    covered = set()
                    while eidx < ne and entries[eidx][1] < whi:
                        s, t = entries[eidx]
                        first = t not in covered
                        covered.add(t)
                        last = (eidx + 1 == ne) or (entries[eidx + 1][1] != t)
                        st = sel_tiles[eidx // SELCH]
                        off = (eidx % SELCH) * WS
                        su = max(s, 0)
                        mt, mo = m_chunks[su // 16]
                        nc.tensor.matmul(
                            out=ps[:, (t % 16) * HS:(t % 16 + 1) * HS],
                            lhsT=st[:, off:off + WS],
                            rhs=mt[:, mo + (su % 16) * HS:mo + (su % 16 + 1) * HS],
                            start=first, stop=last)
                        eidx += 1
                    hp = pool.tile([128, nwin * HS], f32, tag=f"hp{g % 4}")
                    nc.vector.tensor_tensor(
                        out=hp[:], in0=ps[:, :nwin * HS],
                        in1=cb_t[:, :nwin * HS], op=mybir.AluOpType.add)
                    hh = pool.tile([128, nwin * HS], f16, tag=f"hh{g % 4}")
                    nc.scalar.activation(out=hh[:], in_=hp[:],
                                         func=mybir.ActivationFunctionType.Tanh)
                    mc = pool.tile([128, nwin * HS], f16, tag=f"mc{g % 8}")
                    nc.vector.tensor_tensor(
                        out=mc[:], in0=hh[:],
                        in1=sc_t[:, g * 512:g * 512 + nwin * HS],
                        op=mybir.AluOpType.mult)
                    new_chunks.append((mc, 0))
                m_chunks = new_chunks

            # ---- root reduce: sum all m_0 rows ----
            red_t = pool.tile([128, HS], f32, tag="red")
            for g, (mt, mo) in enumerate(m_chunks):
                nblk = min(16, nb[0] - g * 16)
                rc = pool.tile([128, HS], f32, tag="redc")
                ap = mt[:]
                nc.vector.tensor_reduce(
                    out=rc[:],
                    in_=bass.AP(ap.tensor, ap.offset + mo,
                                [[ap.ap[0][0], 128], [1, HS], [HS, nblk]]),
                    axis=mybir.AxisListType.X,
                    op=mybir.AluOpType.add)
                if g == 0:
                    nc.vector.tensor_copy(out=red_t[:], in_=rc[:])
                else:
                    nc.vector.tensor_tensor(out=red_t[:], in0=red_t[:],
                                            in1=rc[:], op=mybir.AluOpType.add)
            rps = psum_pool.tile([128, HS], f32, tag="rp")
            nc.tensor.matmul(out=rps[0:1, :], lhsT=ones_t[:], rhs=red_t[:],
                             start=True, stop=True)
            rout = pool.tile([1, HS], f32, tag="ro")
            nc.vector.tensor_copy(out=rout[:], in_=rps[0:1, :])
            nc.sync.dma_start(out=root_out[:], in_=rout[:])

    nc.finalize()
    return nc


def kernel(embedding, Wx, We, b, parent, etype, levels, is_rel):
    from concourse.bass_utils import run_bass_kernel_spmd

    embedding = np.asarray(embedding, np.float32)
    Wx = np.asarray(Wx, np.float32)
    We = np.asarray(We, np.float32)
    b = np.asarray(b, np.float32)
    parent = np.asarray(parent, np.int64)
    etype = np.asarray(etype, np.int64)
    levels_np = np.asarray(levels, np.int64)
    is_rel = np.asarray(is_rel, bool)

    key = (parent.tobytes(), is_rel.tobytes(), levels_np.tobytes())
    import hashlib
    key = hashlib.sha1(b"".join(key)).hexdigest()
    if key not in _cache:
        layouts, transitions = _build_structure(parent, levels_np, is_rel)
        nc = _compile(layouts, transitions)
        _cache[key] = (layouts, transitions, nc)
    layouts, transitions, nc = _cache[key]

    # ---- numeric inputs ----
    c = embedding @ Wx                       # [H]
    cb_full = c + b[0]                       # [H]
    tanhcb = np.tanh(cb_full)
    WeT = We[:, 0, :]                        # [E, H]

    nbs = [L["nslot"] // 128 for L in layouts]
    in_maps = []
    for core in range(NCORES):
        cs = slice(core * HS, (core + 1) * HS)
        cb_c = cb_full[cs]
        m = {"cb": np.tile(cb_c, (128, 16)).astype(np.float32),
             "ones": np.ones((128, 1), np.float32)}
        for tr in transitions:
            m[f"sel{tr['d']}"] = tr["sel"]
        # scale per level 0..13; m14 for level 14
        for d in range(D - 1):
            L = layouts[d]
            sn = L["slot_node"]
            nb = nbs[d]
            sc = np.zeros((L["nslot"], HS), np.float32)
            real = sn >= 0
            sc[real] = WeT[etype[sn[real]]][:, cs]
            scr = sc.reshape(nb, 128, HS).transpose(1, 0, 2).reshape(128, nb * HS)
            if d == D - 2:
                m["m14"] = (scr * np.tile(tanhcb[cs], nb)[None, :]).astype(np.float16)
            else:
                m[f"scale{d}"] = scr.astype(np.float16)
        in_maps.append(m)

    trace = bool(os.environ.get("CSRNN_TRACE"))
    kw = {}
    if trace:
        import tempfile
        _install_profhook()
        kw = {"trace": True, "tmpdir": tempfile.mkdtemp(prefix="csrnn_")}
    res = run_bass_kernel_spmd(nc, in_maps, list(range(NCORES)), **kw)
    global LAST_EXEC_NS
    LAST_EXEC_NS = res.exec_time_ns
    acc0 = np.concatenate([res.results[core]["root"][0] for core in range(NCORES)])
    root_hidden = np.zeros(H, np.float32) if is_rel[0] else acc0
    out = np.tanh(c + root_hidden + b[0])
    return out[None, :].astype(np.float32)


# revision 7
# speedup vs baseline: 2.0594x; 1.2371x over previous
"""Trainium2 Bass kernel for nn_BasicCSRNN (bottom-up tree RNN).

Strategy: shard H=256 across 8 cores (32 cols each) -> zero cross-core
communication. Per level, the scatter-add to parents becomes ~280 small
TensorEngine matmuls with host-built 0/1 fp16 selection matrices:
sources are sorted by parent slot (layout chosen top-down so every level
uses one consistent layout), each 128-source block hits one 64-slot dst
window. Childless/REL nodes receive zero PSUM contribution and the +cb
bias makes their hidden state tanh(cb) automatically.
"""
import os
import sys

sys.path.insert(0, "/opt/trn_rl_repo")
import numpy as np

D, W = 16, 16384
N = 1 + (D - 1) * W
H, I, E = 256, 256, 16
NCORES = 8
HS = H // NCORES  # 32
WS = 128          # dst window size (slots) = one 128-slot block
PAD_POS = -1      # parentpos for dropped/padding slots (never selected)

_cache = {}
LAST_EXEC_NS = None


def _install_profhook():
    """Register the NTFF profile hook so trace=True works under axon."""
    import types
    try:
        from antenv import axon_hooks  # noqa: F401
        return
    except ImportError:
        pass
    import antenv
    mod = types.ModuleType("antenv.axon_hooks")
    _hook = [None]
    mod.set_axon_ntff_profile_hook = lambda h: _hook.__setitem__(0, h)
    mod.get_axon_ntff_profile_hook = lambda: _hook[0]
    sys.modules["antenv.axon_hooks"] = mod
    antenv.axon_hooks = mod
    from trn_agent_boot.trn_boot import _ntff_profile_via_ctypes
    mod.set_axon_ntff_profile_hook(
        _ntff_profile_via_ctypes("/opt/axon/libaxon_pjrt.so"))
    import concourse.bass_utils as bu
    bu.upload_artifacts = lambda tmpdir: "local://" + str(tmpdir)


def _build_structure(parent, levels, is_rel):
    """Host-side layout build. Returns per-level slot layouts, window/entry
    lists and the fp16 selection-matrix streams (core-independent)."""
    lv = [np.asarray(levels[d], np.int64) for d in range(D - 1)]
    # children counts per node (from each level's parents)
    cnt = np.zeros(N, np.int64)
    for d in range(D - 1):
        np.add.at(cnt, parent[lv[d]], 1)

    slotpos = np.full(N, -1, np.int64)   # node -> slot within its level
    layouts = []  # per level d: dict(slot_node: [NSLOT] node id or -1)

    # ---- level 0 layout: arbitrary order, pad to 128 ----
    nodes0 = lv[0]
    nslot0 = ((len(nodes0) + 127) // 128) * 128
    slot_node0 = np.full(nslot0, -1, np.int64)
    slot_node0[:len(nodes0)] = nodes0
    slotpos[nodes0] = np.arange(len(nodes0))
    layouts.append({"slot_node": slot_node0, "nslot": nslot0})

    # ---- levels 1..14: group by parent window, bin-pack into 128-blocks ----
    for d in range(1, D - 1):
        nodes = lv[d]
        par = parent[nodes]
        dropped = is_rel[par]            # children of REL parents: no sel row
        ppos = slotpos[par]
        win = ppos // WS                 # parent window id
        # order: non-dropped sorted by window, then dropped (fake window = big)
        wkey = np.where(dropped, 1 << 40, win)
        order = np.argsort(wkey, kind="stable")
        snodes = nodes[order]
        swin = wkey[order]
        # bin-pack: consecutive whole windows into 128-slot blocks; a window's
        # nodes must not straddle a block (unless window itself > 128).
        slot_node = []
        i = 0
        nnodes = len(snodes)
        cur = 0  # slots used in current block
        while i < nnodes:
            j = i + 1
            while j < nnodes and swin[j] == swin[i]:
                j += 1
            g = j - i  # nodes in this window
            if g > 128 - cur and cur > 0:
                slot_node += [-1] * (128 - cur)   # pad out block
                cur = 0
            take = i
            while g > 128:
                # oversized window: split across dedicated blocks
                slot_node += list(snodes[take:take + 128])
                take += 128
                g -= 128
                cur = 0
            slot_node += list(snodes[take:j])
            cur = (cur + g) % 128
            i = j
        if len(slot_node) % 128:
            slot_node += [-1] * (128 - len(slot_node) % 128)
        slot_node = np.array(slot_node, np.int64)
        nslot = len(slot_node)
        real = slot_node >= 0
        slotpos[slot_node[real]] = np.nonzero(real)[0]
        layouts.append({"slot_node": slot_node, "nslot": nslot})

    # ---- per-transition (d -> d-1) entries and sel streams ----
    transitions = []  # d=1..14: dict(entries, sel, nb, nw_prev)
    for d in range(1, D - 1):
        L = layouts[d]
        slot_node = L["slot_node"]
        nb = L["nslot"] // 128
        nw_prev = layouts[d - 1]["nslot"] // WS
        # per slot: parentpos (or PAD)
        sp = np.full(L["nslot"], PAD_POS, np.int64)
        real = slot_node >= 0
        rn = slot_node[real]
        keep = ~is_rel[parent[rn]]
        idx = np.nonzero(real)[0][keep]
        sp[idx] = slotpos[parent[slot_node[idx]]]
        # entries: (block s, window t) for windows present in block
        entries = []  # list of (s, t)
        win_of = np.where(sp >= 0, sp // WS, -1).reshape(nb, 128)
        for s in range(nb):
            ws_here = np.unique(win_of[s])
            for t in ws_here:
                if t >= 0:
                    entries.append((s, int(t)))
        # every window needs at least one entry (else PSUM slice is never
        # written); empty windows get a dummy all-zero sel entry
        have = {t for _, t in entries}
        for t in range(nw_prev):
            if t not in have:
                entries.append((-1, t))
        # order entries by (t, s) so same-window entries are adjacent
        entries.sort(key=lambda e: (e[1], e[0]))
        ne = len(entries)
        import ml_dtypes
        sel = np.zeros((128, ne * WS), ml_dtypes.float8_e4m3)
        spb = sp.reshape(nb, 128)
        for e, (s, t) in enumerate(entries):
            if s < 0:
                continue
            rows = spb[s]
            k = np.nonzero((rows >= t * WS) & (rows < (t + 1) * WS))[0]
            sel[k, e * WS + (rows[k] - t * WS)] = 1.0
        transitions.append({"entries": entries, "sel": sel, "nb": nb,
                            "nw_prev": nw_prev, "d": d})
    return layouts, transitions


def _compile(layouts, transitions):
    import concourse.bass as bass
    import concourse.bacc as bacc
    import concourse.mybir as mybir
    import concourse.tile as tile

    f32 = mybir.dt.float32
    f16 = mybir.dt.float16
    f8 = mybir.dt.float8e4

    nc = bacc.Bacc("TRN2", target_bir_lowering=False, debug=False,
                   num_devices=NCORES)
    nb = [L["nslot"] // 128 for L in layouts]
    ng = [(b + 15) // 16 for b in nb]
    m14_in = nc.dram_tensor("m14", [128, nb[14] * HS], f16, kind="ExternalInput")
    cb_in = nc.dram_tensor("cb", [128, 512], f32, kind="ExternalInput")
    sel_in = {}
    for tr in transitions:
        d = tr["d"]
        ne = len(tr["entries"])
        sel_in[d] = nc.dram_tensor(f"sel{d}", [128, ne * WS], f8,
                                   kind="ExternalInput")
    scale_in = {d: nc.dram_tensor(f"scale{d}", [128, nb[d] * HS], f16,
                                  kind="ExternalInput")
                for d in range(D - 2)}  # levels 0..13 (level 14 folded in m14)
    ones_in = nc.dram_tensor("ones", [128, 1], f32, kind="ExternalInput")
    root_out = nc.dram_tensor("root", [1, HS], f32, kind="ExternalOutput")

    SELCH = 24  # sel entries per DMA chunk

    with tile.TileContext(nc) as tc:
        with tc.tile_pool(name="const", bufs=1) as cpool, \
             tc.tile_pool(name="work", bufs=2) as pool, \
             tc.tile_pool(name="selp", bufs=6) as selpool, \
             tc.tile_pool(name="psum", bufs=5, space="PSUM") as psum_pool:
            cb_t = cpool.tile([128, 512], f32, tag="cb")
            nc.sync.dma_start(out=cb_t[:], in_=cb_in[:])
            ones_t = cpool.tile([128, 1], f32, tag="ones")
            nc.sync.dma_start(out=ones_t[:], in_=ones_in[:])

            m14_t = cpool.tile([128, nb[14] * HS], f16, tag="m14")
            nc.sync.dma_start(out=m14_t[:], in_=m14_in[:])
            # m chunks: list of (tile, elem offset) per 16-block group
            m_chunks = [(m14_t, 512 * g) for g in range(ng[14])]

            for tr in reversed(transitions):   # d = 14 .. 1
                d = tr["d"]
                dd = d - 1                      # destination level
                entries = tr["entries"]
                ne = len(entries)
                nwd = nb[dd]                    # windows = blocks of dest level
                sel_tiles = []
                for c in range(0, ne, SELCH):
                    hi = min(c + SELCH, ne)
                    st = selpool.tile([128, SELCH * WS], f8, tag="sel")
                    nc.sync.dma_start(out=st[:, :(hi - c) * WS],
                                      in_=sel_in[d][:, c * WS:hi * WS])
                    sel_tiles.append(st)
                sc_t = pool.tile([128, nb[dd] * HS], f16, tag="scale")
                nc.sync.dma_start(out=sc_t[:], in_=scale_in[dd][:])

                new_chunks = []
                eidx = 0
                for g in range(ng[dd]):
                    wlo, whi = g * 16, min((g + 1) * 16, nwd)
                    nwin = whi - wlo
                    ps = psum_pool.tile([128, 512], f32, tag="ps")
                    covered = set()
                    while eidx < ne and entries[eidx][1] < whi:
                        s, t = entries[eidx]
                        first = t not in covered
                        covered.add(t)
                        last = (eidx + 1 == ne) or (entries[eidx + 1][1] != t)
                        st = sel_tiles[eidx // SELCH]
                        off = (eidx % SELCH) * WS
                        su = max(s, 0)
                        mt, mo = m_chunks[su // 16]
                        nc.tensor.matmul(
                            out=ps[:, (t % 16) * HS:(t % 16 + 1) * HS],
                            lhsT=st[:, off:off + WS],
                            rhs=mt[:, mo + (su % 16) * HS:mo + (su % 16 + 1) * HS],
                            start=first, stop=last)
                        eidx += 1
                    hp = pool.tile([128, nwin * HS], f32, tag=f"hp{g % 4}")
                    nc.vector.tensor_tensor(
                        out=hp[:], in0=ps[:, :nwin * HS],
                        in1=cb_t[:, :nwin * HS], op=mybir.AluOpType.add)
                    hh = pool.tile([128, nwin * HS], f16, tag=f"hh{g % 4}")
                    nc.scalar.activation(out=hh[:], in_=hp[:],
                                         func=mybir.ActivationFunctionType.Tanh)
                    mc = pool.tile([128, nwin * HS], f16, tag=f"mc{g % 8}")
                    nc.vector.tensor_tensor(
                        out=mc[:], in0=hh[:],
                        in1=sc_t[:, g * 512:g * 512 + nwin * HS],
                        op=mybir.AluOpType.mult)
                    new_chunks.append((mc, 0))
                m_chunks = new_chunks

            # ---- root reduce: sum all m_0 rows ----
            red_t = pool.tile([128, HS], f32, tag="red")
            for g, (mt, mo) in enumerate(m_chunks):
                nblk = min(16, nb[0] - g * 16)
                rc = pool.tile([128, HS], f32, tag="redc")
                ap = mt[:]
                nc.vector.tensor_reduce(
                    out=rc[:],
                    in_=bass.AP(ap.tensor, ap.offset + mo,
                                [[ap.ap[0][0], 128], [1, HS], [HS, nblk]]),
                    axis=mybir.AxisListType.X,
                    op=mybir.AluOpType.add)
                if g == 0:
                    nc.vector.tensor_copy(out=red_t[:], in_=rc[:])
                else:
                    nc.vector.tensor_tensor(out=red_t[:], in0=red_t[:],
                                            in1=rc[:], op=mybir.AluOpType.add)
            rps = psum_pool.tile([128, HS], f32, tag="ps")
            nc.tensor.matmul(out=rps[0:1, :], lhsT=ones_t[:], rhs=red_t[:],
                             start=True, stop=True)
            rout = pool.tile([1, HS], f32, tag="ro")
            nc.vector.tensor_copy(out=rout[:], in_=rps[0:1, :])
            nc.sync.dma_start(out=root_out[:], in_=rout[:])

    nc.finalize()
    return nc


def kernel(embedding, Wx, We, b, parent, etype, levels, is_rel):
    from concourse.bass_utils import run_bass_kernel_spmd

    embedding = np.asarray(embedding, np.float32)
    Wx = np.asarray(Wx, np.float32)
    We = np.asarray(We, np.float32)
    b = np.asarray(b, np.float32)
    parent = np.asarray(parent, np.int64)
    etype = np.asarray(etype, np.int64)
    levels_np = np.asarray(levels, np.int64)
    is_rel = np.asarray(is_rel, bool)

    key = (parent.tobytes(), is_rel.tobytes(), levels_np.tobytes())
    import hashlib
    key = hashlib.sha1(b"".join(key)).hexdigest()
    if key not in _cache:
        layouts, transitions = _build_structure(parent, levels_np, is_rel)
        nc = _compile(layouts, transitions)
        _cache[key] = (layouts, transitions, nc)
    layouts, transitions, nc = _cache[key]

    # ---- numeric inputs ----
    c = embedding @ Wx                       # [H]
    cb_full = c + b[0]                       # [H]
    tanhcb = np.tanh(cb_full)
    WeT = We[:, 0, :]                        # [E, H]

    nbs = [L["nslot"] // 128 for L in layouts]
    in_maps = []
    for core in range(NCORES):
        cs = slice(core * HS, (core + 1) * HS)
        cb_c = cb_full[cs]
        m = {"cb": np.tile(cb_c, (128, 16)).astype(np.float32),
             "ones": np.ones((128, 1), np.float32)}
        for tr in transitions:
            m[f"sel{tr['d']}"] = tr["sel"]
        # scale per level 0..13; m14 for level 14
        for d in range(D - 1):
            L = layouts[d]
            sn = L["slot_node"]
            nb = nbs[d]
            sc = np.zeros((L["nslot"], HS), np.float32)
            real = sn >= 0
            sc[real] = WeT[etype[sn[real]]][:, cs]
            scr = sc.reshape(nb, 128, HS).transpose(1, 0, 2).reshape(128, nb * HS)
            if d == D - 2:
                m["m14"] = (scr * np.tile(tanhcb[cs], nb)[None, :]).astype(np.float16)
            else:
                m[f"scale{d}"] = scr.astype(np.float16)
        in_maps.append(m)

    trace = bool(os.environ.get("CSRNN_TRACE"))
    kw = {}
    if trace:
        import tempfile
        _install_profhook()
        kw = {"trace": True, "tmpdir": tempfile.mkdtemp(prefix="csrnn_")}
    res = run_bass_kernel_spmd(nc, in_maps, list(range(NCORES)), **kw)
    global LAST_EXEC_NS
    LAST_EXEC_NS = res.exec_time_ns
    acc0 = np.concatenate([res.results[core]["root"][0] for core in range(NCORES)])
    root_hidden = np.zeros(H, np.float32) if is_rel[0] else acc0
    out = np.tanh(c + root_hidden + b[0])
    return out[None, :].astype(np.float32)


# revision 8
# speedup vs baseline: 2.0918x; 1.0158x over previous
"""Trainium2 Bass kernel for nn_BasicCSRNN (bottom-up tree RNN).

Strategy: shard H=256 across 8 cores (32 cols each) -> zero cross-core
communication. Per level, the scatter-add to parents becomes ~280 small
TensorEngine matmuls with host-built 0/1 fp16 selection matrices:
sources are sorted by parent slot (layout chosen top-down so every level
uses one consistent layout), each 128-source block hits one 64-slot dst
window. Childless/REL nodes receive zero PSUM contribution and the +cb
bias makes their hidden state tanh(cb) automatically.
"""
import os
import sys

sys.path.insert(0, "/opt/trn_rl_repo")
import numpy as np

D, W = 16, 16384
N = 1 + (D - 1) * W
H, I, E = 256, 256, 16
NCORES = 8
HS = H // NCORES  # 32
WS = 128          # dst window size (slots) = one 128-slot block
PAD_POS = -1      # parentpos for dropped/padding slots (never selected)

_cache = {}
LAST_EXEC_NS = None


def _install_profhook():
    """Register the NTFF profile hook so trace=True works under axon."""
    import types
    try:
        from antenv import axon_hooks  # noqa: F401
        return
    except ImportError:
        pass
    import antenv
    mod = types.ModuleType("antenv.axon_hooks")
    _hook = [None]
    mod.set_axon_ntff_profile_hook = lambda h: _hook.__setitem__(0, h)
    mod.get_axon_ntff_profile_hook = lambda: _hook[0]
    sys.modules["antenv.axon_hooks"] = mod
    antenv.axon_hooks = mod
    from trn_agent_boot.trn_boot import _ntff_profile_via_ctypes
    mod.set_axon_ntff_profile_hook(
        _ntff_profile_via_ctypes("/opt/axon/libaxon_pjrt.so"))
    import concourse.bass_utils as bu
    bu.upload_artifacts = lambda tmpdir: "local://" + str(tmpdir)


def _build_structure(parent, levels, is_rel):
    """Host-side layout build. Returns per-level slot layouts, window/entry
    lists and the fp16 selection-matrix streams (core-independent)."""
    lv = [np.asarray(levels[d], np.int64) for d in range(D - 1)]
    # children counts per node (from each level's parents)
    cnt = np.zeros(N, np.int64)
    for d in range(D - 1):
        np.add.at(cnt, parent[lv[d]], 1)

    slotpos = np.full(N, -1, np.int64)   # node -> slot within its level
    layouts = []  # per level d: dict(slot_node: [NSLOT] node id or -1)

    # ---- level 0 layout: arbitrary order, pad to 128 ----
    nodes0 = lv[0]
    nslot0 = ((len(nodes0) + 127) // 128) * 128
    slot_node0 = np.full(nslot0, -1, np.int64)
    slot_node0[:len(nodes0)] = nodes0
    slotpos[nodes0] = np.arange(len(nodes0))
    layouts.append({"slot_node": slot_node0, "nslot": nslot0})

    # ---- levels 1..14: group by parent window, bin-pack into 128-blocks ----
    for d in range(1, D - 1):
        nodes = lv[d]
        par = parent[nodes]
        dropped = is_rel[par]            # children of REL parents: no sel row
        ppos = slotpos[par]
        win = ppos // WS                 # parent window id
        # order: non-dropped sorted by window, then dropped (fake window = big)
        wkey = np.where(dropped, 1 << 40, win)
        order = np.argsort(wkey, kind="stable")
        snodes = nodes[order]
        swin = wkey[order]
        # bin-pack: consecutive whole windows into 128-slot blocks; a window's
        # nodes must not straddle a block (unless window itself > 128).
        slot_node = []
        i = 0
        nnodes = len(snodes)
        cur = 0  # slots used in current block
        while i < nnodes:
            j = i + 1
            while j < nnodes and swin[j] == swin[i]:
                j += 1
            g = j - i  # nodes in this window
            if g > 128 - cur and cur > 0:
                slot_node += [-1] * (128 - cur)   # pad out block
                cur = 0
            take = i
            while g > 128:
                # oversized window: split across dedicated blocks
                slot_node += list(snodes[take:take + 128])
                take += 128
                g -= 128
                cur = 0
            slot_node += list(snodes[take:j])
            cur = (cur + g) % 128
            i = j
        if len(slot_node) % 128:
            slot_node += [-1] * (128 - len(slot_node) % 128)
        slot_node = np.array(slot_node, np.int64)
        nslot = len(slot_node)
        real = slot_node >= 0
        slotpos[slot_node[real]] = np.nonzero(real)[0]
        layouts.append({"slot_node": slot_node, "nslot": nslot})

    # ---- per-transition (d -> d-1) entries and sel streams ----
    transitions = []  # d=1..14: dict(entries, sel, nb, nw_prev)
    for d in range(1, D - 1):
        L = layouts[d]
        slot_node = L["slot_node"]
        nb = L["nslot"] // 128
        nw_prev = layouts[d - 1]["nslot"] // WS
        # per slot: parentpos (or PAD)
        sp = np.full(L["nslot"], PAD_POS, np.int64)
        real = slot_node >= 0
        rn = slot_node[real]
        keep = ~is_rel[parent[rn]]
        idx = np.nonzero(real)[0][keep]
        sp[idx] = slotpos[parent[slot_node[idx]]]
        # entries: (block s, window t) for windows present in block
        entries = []  # list of (s, t)
        win_of = np.where(sp >= 0, sp // WS, -1).reshape(nb, 128)
        for s in range(nb):
            ws_here = np.unique(win_of[s])
            for t in ws_here:
                if t >= 0:
                    entries.append((s, int(t)))
        # every window needs at least one entry (else PSUM slice is never
        # written); empty windows get a dummy all-zero sel entry
        have = {t for _, t in entries}
        for t in range(nw_prev):
            if t not in have:
                entries.append((-1, t))
        # order entries by (t, s) so same-window entries are adjacent
        entries.sort(key=lambda e: (e[1], e[0]))
        ne = len(entries)
        import ml_dtypes
        sel = np.zeros((128, ne * WS), ml_dtypes.float8_e4m3)
        spb = sp.reshape(nb, 128)
        for e, (s, t) in enumerate(entries):
            if s < 0:
                continue
            rows = spb[s]
            k = np.nonzero((rows >= t * WS) & (rows < (t + 1) * WS))[0]
            sel[k, e * WS + (rows[k] - t * WS)] = 1.0
        transitions.append({"entries": entries, "sel": sel, "nb": nb,
                            "nw_prev": nw_prev, "d": d})
    return layouts, transitions


def _compile(layouts, transitions):
    import concourse.bass as bass
    import concourse.bacc as bacc
    import concourse.mybir as mybir
    import concourse.tile as tile

    f32 = mybir.dt.float32
    f16 = mybir.dt.float16
    f8 = mybir.dt.float8e4

    nc = bacc.Bacc("TRN2", target_bir_lowering=False, debug=False,
                   num_devices=NCORES)
    nb = [L["nslot"] // 128 for L in layouts]
    ng = [(b + 15) // 16 for b in nb]
    m14_in = nc.dram_tensor("m14", [128, nb[14] * HS], f16, kind="ExternalInput")
    cb_in = nc.dram_tensor("cb", [128, 512], f32, kind="ExternalInput")
    sel_in = {}
    for tr in transitions:
        d = tr["d"]
        ne = len(tr["entries"])
        sel_in[d] = nc.dram_tensor(f"sel{d}", [128, ne * WS], f8,
                                   kind="ExternalInput")
    scale_in = {d: nc.dram_tensor(f"scale{d}", [128, nb[d] * HS], f16,
                                  kind="ExternalInput")
                for d in range(D - 2)}  # levels 0..13 (level 14 folded in m14)
    ones_in = nc.dram_tensor("ones", [128, 1], f32, kind="ExternalInput")
    root_out = nc.dram_tensor("root", [1, HS], f32, kind="ExternalOutput")

    SELCH = 32  # sel entries per DMA chunk

    with tile.TileContext(nc) as tc:
        with tc.tile_pool(name="const", bufs=1) as cpool, \
             tc.tile_pool(name="work", bufs=2) as pool, \
             tc.tile_pool(name="selp", bufs=8) as selpool, \
             tc.tile_pool(name="psum", bufs=5, space="PSUM") as psum_pool:
            cb_t = cpool.tile([128, 512], f32, tag="cb")
            nc.sync.dma_start(out=cb_t[:], in_=cb_in[:])
            ones_t = cpool.tile([128, 1], f32, tag="ones")
            nc.sync.dma_start(out=ones_t[:], in_=ones_in[:])

            m14_t = cpool.tile([128, nb[14] * HS], f16, tag="m14")
            nc.sync.dma_start(out=m14_t[:], in_=m14_in[:])
            # m chunks: list of (tile, elem offset) per 16-block group
            m_chunks = [(m14_t, 512 * g) for g in range(ng[14])]

            for tr in reversed(transitions):   # d = 14 .. 1
                d = tr["d"]
                dd = d - 1                      # destination level
                entries = tr["entries"]
                ne = len(entries)
                nwd = nb[dd]                    # windows = blocks of dest level
                sel_tiles = []
                for c in range(0, ne, SELCH):
                    hi = min(c + SELCH, ne)
                    st = selpool.tile([128, SELCH * WS], f8, tag="sel")
                    nc.sync.dma_start(out=st[:, :(hi - c) * WS],
                                      in_=sel_in[d][:, c * WS:hi * WS])
                    sel_tiles.append(st)
                sc_t = pool.tile([128, nb[dd] * HS], f16, tag="scale")
                nc.sync.dma_start(out=sc_t[:], in_=scale_in[dd][:])

                new_chunks = []
                eidx = 0
                for g in range(ng[dd]):
                    wlo, whi = g * 16, min((g + 1) * 16, nwd)
                    nwin = whi - wlo
                    ps = psum_pool.tile([128, 512], f32, tag="ps")
                    covered = set()
                    while eidx < ne and entries[eidx][1] < whi:
                        s, t = entries[eidx]
                        first = t not in covered
                        covered.add(t)
                        last = (eidx + 1 == ne) or (entries[eidx + 1][1] != t)
                        st = sel_tiles[eidx // SELCH]
                        off = (eidx % SELCH) * WS
                        su = max(s, 0)
                        mt, mo = m_chunks[su // 16]
                        nc.tensor.matmul(
                            out=ps[:, (t % 16) * HS:(t % 16 + 1) * HS],
                            lhsT=st[:, off:off + WS],
                            rhs=mt[:, mo + (su % 16) * HS:mo + (su % 16 + 1) * HS],
                            start=first, stop=last)
                        eidx += 1
                    hp = pool.tile([128, nwin * HS], f32, tag=f"hp{g % 4}")
                    nc.vector.tensor_tensor(
                        out=hp[:], in0=ps[:, :nwin * HS],
                        in1=cb_t[:, :nwin * HS], op=mybir.AluOpType.add)
                    hh = pool.tile([128, nwin * HS], f16, tag=f"hh{g % 4}")
                    nc.scalar.activation(out=hh[:], in_=hp[:],
                                         func=mybir.ActivationFunctionType.Tanh)
                    mc = pool.tile([128, nwin * HS], f16, tag=f"mc{g % 8}")
                    nc.vector.tensor_tensor(
                        out=mc[:], in0=hh[:],
                        in1=sc_t[:, g * 512:g * 512 + nwin * HS],
                        op=mybir.AluOpType.mult)
                    new_chunks.append((mc, 0))
                m_chunks = new_chunks

            # ---- root reduce: sum all m_0 rows ----
            red_t = pool.tile([128, HS], f32, tag="red")
            for g, (mt, mo) in enumerate(m_chunks):
                nblk = min(16, nb[0] - g * 16)
                rc = pool.tile([128, HS], f32, tag="redc")
                ap = mt[:]
                nc.vector.tensor_reduce(
                    out=rc[:],
                    in_=bass.AP(ap.tensor, ap.offset + mo,
                                [[ap.ap[0][0], 128], [1, HS], [HS, nblk]]),
                    axis=mybir.AxisListType.X,
                    op=mybir.AluOpType.add)
                if g == 0:
                    nc.vector.tensor_copy(out=red_t[:], in_=rc[:])
                else:
                    nc.vector.tensor_tensor(out=red_t[:], in0=red_t[:],
                                            in1=rc[:], op=mybir.AluOpType.add)
            rps = psum_pool.tile([128, HS], f32, tag="ps")
            nc.tensor.matmul(out=rps[0:1, :], lhsT=ones_t[:], rhs=red_t[:],
                             start=True, stop=True)
            rout = pool.tile([1, HS], f32, tag="ro")
            nc.vector.tensor_copy(out=rout[:], in_=rps[0:1, :])
            nc.sync.dma_start(out=root_out[:], in_=rout[:])

    nc.finalize()
    return nc


def kernel(embedding, Wx, We, b, parent, etype, levels, is_rel):
    from concourse.bass_utils import run_bass_kernel_spmd

    embedding = np.asarray(embedding, np.float32)
    Wx = np.asarray(Wx, np.float32)
    We = np.asarray(We, np.float32)
    b = np.asarray(b, np.float32)
    parent = np.asarray(parent, np.int64)
    etype = np.asarray(etype, np.int64)
    levels_np = np.asarray(levels, np.int64)
    is_rel = np.asarray(is_rel, bool)

    key = (parent.tobytes(), is_rel.tobytes(), levels_np.tobytes())
    import hashlib
    key = hashlib.sha1(b"".join(key)).hexdigest()
    if key not in _cache:
        layouts, transitions = _build_structure(parent, levels_np, is_rel)
        nc = _compile(layouts, transitions)
        _cache[key] = (layouts, transitions, nc)
    layouts, transitions, nc = _cache[key]

    # ---- numeric inputs ----
    c = embedding @ Wx                       # [H]
    cb_full = c + b[0]                       # [H]
    tanhcb = np.tanh(cb_full)
    WeT = We[:, 0, :]                        # [E, H]

    nbs = [L["nslot"] // 128 for L in layouts]
    in_maps = []
    for core in range(NCORES):
        cs = slice(core * HS, (core + 1) * HS)
        cb_c = cb_full[cs]
        m = {"cb": np.tile(cb_c, (128, 16)).astype(np.float32),
             "ones": np.ones((128, 1), np.float32)}
        for tr in transitions:
            m[f"sel{tr['d']}"] = tr["sel"]
        # scale per level 0..13; m14 for level 14
        for d in range(D - 1):
            L = layouts[d]
            sn = L["slot_node"]
            nb = nbs[d]
            sc = np.zeros((L["nslot"], HS), np.float32)
            real = sn >= 0
            sc[real] = WeT[etype[sn[real]]][:, cs]
            scr = sc.reshape(nb, 128, HS).transpose(1, 0, 2).reshape(128, nb * HS)
            if d == D - 2:
                m["m14"] = (scr * np.tile(tanhcb[cs], nb)[None, :]).astype(np.float16)
            else:
                m[f"scale{d}"] = scr.astype(np.float16)
        in_maps.append(m)

    trace = bool(os.environ.get("CSRNN_TRACE"))
    kw = {}
    if trace:
        import tempfile
        _install_profhook()
        kw = {"trace": True, "tmpdir": tempfile.mkdtemp(prefix="csrnn_")}
    res = run_bass_kernel_spmd(nc, in_maps, list(range(NCORES)), **kw)
    global LAST_EXEC_NS
    LAST_EXEC_NS = res.exec_time_ns
    acc0 = np.concatenate([res.results[core]["root"][0] for core in range(NCORES)])
    root_hidden = np.zeros(H, np.float32) if is_rel[0] else acc0
    out = np.tanh(c + root_hidden + b[0])
    return out[None, :].astype(np.float32)


# revision 9
# speedup vs baseline: 2.1420x; 1.0240x over previous
"""Trainium2 Bass kernel for nn_BasicCSRNN (bottom-up tree RNN).

Strategy: shard H=256 across 8 cores (32 cols each) -> zero cross-core
communication. Per level, the scatter-add to parents becomes ~280 small
TensorEngine matmuls with host-built 0/1 fp16 selection matrices:
sources are sorted by parent slot (layout chosen top-down so every level
uses one consistent layout), each 128-source block hits one 64-slot dst
window. Childless/REL nodes receive zero PSUM contribution and the +cb
bias makes their hidden state tanh(cb) automatically.
"""
import os
import sys

sys.path.insert(0, "/opt/trn_rl_repo")
import numpy as np

D, W = 16, 16384
N = 1 + (D - 1) * W
H, I, E = 256, 256, 16
NCORES = 8
HS = H // NCORES  # 32
WS = 128          # dst window size (slots) = one 128-slot block
PAD_POS = -1      # parentpos for dropped/padding slots (never selected)

_cache = {}
LAST_EXEC_NS = None


def _install_profhook():
    """Register the NTFF profile hook so trace=True works under axon."""
    import types
    try:
        from antenv import axon_hooks  # noqa: F401
        return
    except ImportError:
        pass
    import antenv
    mod = types.ModuleType("antenv.axon_hooks")
    _hook = [None]
    mod.set_axon_ntff_profile_hook = lambda h: _hook.__setitem__(0, h)
    mod.get_axon_ntff_profile_hook = lambda: _hook[0]
    sys.modules["antenv.axon_hooks"] = mod
    antenv.axon_hooks = mod
    from trn_agent_boot.trn_boot import _ntff_profile_via_ctypes
    mod.set_axon_ntff_profile_hook(
        _ntff_profile_via_ctypes("/opt/axon/libaxon_pjrt.so"))
    import concourse.bass_utils as bu
    bu.upload_artifacts = lambda tmpdir: "local://" + str(tmpdir)


def _build_structure(parent, levels, is_rel):
    """Host-side layout build. Returns per-level slot layouts, window/entry
    lists and the fp16 selection-matrix streams (core-independent)."""
    lv = [np.asarray(levels[d], np.int64) for d in range(D - 1)]
    # children counts per node (from each level's parents)
    cnt = np.zeros(N, np.int64)
    for d in range(D - 1):
        np.add.at(cnt, parent[lv[d]], 1)

    slotpos = np.full(N, -1, np.int64)   # node -> slot within its level
    layouts = []  # per level d: dict(slot_node: [NSLOT] node id or -1)

    # ---- level 0 layout: arbitrary order, pad to 128 ----
    nodes0 = lv[0]
    nslot0 = ((len(nodes0) + 127) // 128) * 128
    slot_node0 = np.full(nslot0, -1, np.int64)
    slot_node0[:len(nodes0)] = nodes0
    slotpos[nodes0] = np.arange(len(nodes0))
    layouts.append({"slot_node": slot_node0, "nslot": nslot0})

    # ---- levels 1..14: group by parent window, bin-pack into 128-blocks ----
    for d in range(1, D - 1):
        nodes = lv[d]
        par = parent[nodes]
        dropped = is_rel[par]            # children of REL parents: no sel row
        ppos = slotpos[par]
        win = ppos // WS                 # parent window id
        # order: non-dropped sorted by window, then dropped (fake window = big)
        wkey = np.where(dropped, 1 << 40, win)
        order = np.argsort(wkey, kind="stable")
        snodes = nodes[order]
        swin = wkey[order]
        # bin-pack: consecutive whole windows into 128-slot blocks; a window's
        # nodes must not straddle a block (unless window itself > 128).
        slot_node = []
        i = 0
        nnodes = len(snodes)
        cur = 0  # slots used in current block
        while i < nnodes:
            j = i + 1
            while j < nnodes and swin[j] == swin[i]:
                j += 1
            g = j - i  # nodes in this window
            if g > 128 - cur and cur > 0:
                slot_node += [-1] * (128 - cur)   # pad out block
                cur = 0
            take = i
            while g > 128:
                # oversized window: split across dedicated blocks
                slot_node += list(snodes[take:take + 128])
                take += 128
                g -= 128
                cur = 0
            slot_node += list(snodes[take:j])
            cur = (cur + g) % 128
            i = j
        if len(slot_node) % 128:
            slot_node += [-1] * (128 - len(slot_node) % 128)
        slot_node = np.array(slot_node, np.int64)
        nslot = len(slot_node)
        real = slot_node >= 0
        slotpos[slot_node[real]] = np.nonzero(real)[0]
        layouts.append({"slot_node": slot_node, "nslot": nslot})

    # ---- per-transition (d -> d-1) entries and sel streams ----
    transitions = []  # d=1..14: dict(entries, sel, nb, nw_prev)
    for d in range(1, D - 1):
        L = layouts[d]
        slot_node = L["slot_node"]
        nb = L["nslot"] // 128
        nw_prev = layouts[d - 1]["nslot"] // WS
        # per slot: parentpos (or PAD)
        sp = np.full(L["nslot"], PAD_POS, np.int64)
        real = slot_node >= 0
        rn = slot_node[real]
        keep = ~is_rel[parent[rn]]
        idx = np.nonzero(real)[0][keep]
        sp[idx] = slotpos[parent[slot_node[idx]]]
        # entries: (block s, window t) for windows present in block
        entries = []  # list of (s, t)
        win_of = np.where(sp >= 0, sp // WS, -1).reshape(nb, 128)
        for s in range(nb):
            ws_here = np.unique(win_of[s])
            for t in ws_here:
                if t >= 0:
                    entries.append((s, int(t)))
        # empty windows get their PSUM slice zeroed on-device instead of a
        # dummy matmul entry
        have = {t for _, t in entries}
        empties = [t for t in range(nw_prev) if t not in have]
        # order entries by (t, s) so same-window entries are adjacent
        entries.sort(key=lambda e: (e[1], e[0]))
        ne = len(entries)
        import ml_dtypes
        sel = np.zeros((128, ne * WS), ml_dtypes.float8_e4m3)
        spb = sp.reshape(nb, 128)
        for e, (s, t) in enumerate(entries):
            rows = spb[s]
            k = np.nonzero((rows >= t * WS) & (rows < (t + 1) * WS))[0]
            sel[k, e * WS + (rows[k] - t * WS)] = 1.0
        transitions.append({"entries": entries, "sel": sel, "nb": nb,
                            "nw_prev": nw_prev, "d": d, "empties": empties})
    return layouts, transitions


def _compile(layouts, transitions):
    import concourse.bass as bass
    import concourse.bacc as bacc
    import concourse.mybir as mybir
    import concourse.tile as tile

    f32 = mybir.dt.float32
    f16 = mybir.dt.float16
    f8 = mybir.dt.float8e4

    nc = bacc.Bacc("TRN2", target_bir_lowering=False, debug=False,
                   num_devices=NCORES)
    nb = [L["nslot"] // 128 for L in layouts]
    ng = [(b + 15) // 16 for b in nb]
    m14_in = nc.dram_tensor("m14", [128, nb[14] * HS], f16, kind="ExternalInput")
    cb_in = nc.dram_tensor("cb", [128, 512], f32, kind="ExternalInput")
    sel_in = {}
    for tr in transitions:
        d = tr["d"]
        ne = len(tr["entries"])
        sel_in[d] = nc.dram_tensor(f"sel{d}", [128, ne * WS], f8,
                                   kind="ExternalInput")
    scale_in = {d: nc.dram_tensor(f"scale{d}", [128, nb[d] * HS], f16,
                                  kind="ExternalInput")
                for d in range(D - 2)}  # levels 0..13 (level 14 folded in m14)
    ones_in = nc.dram_tensor("ones", [128, 1], f32, kind="ExternalInput")
    root_out = nc.dram_tensor("root", [1, HS], f32, kind="ExternalOutput")

    SELCH = 32  # sel entries per DMA chunk

    with tile.TileContext(nc) as tc:
        with tc.tile_pool(name="const", bufs=1) as cpool, \
             tc.tile_pool(name="work", bufs=2) as pool, \
             tc.tile_pool(name="selp", bufs=8) as selpool, \
             tc.tile_pool(name="psum", bufs=5, space="PSUM") as psum_pool:
            cb_t = cpool.tile([128, 512], f32, tag="cb")
            nc.sync.dma_start(out=cb_t[:], in_=cb_in[:])
            ones_t = cpool.tile([128, 1], f32, tag="ones")
            nc.sync.dma_start(out=ones_t[:], in_=ones_in[:])

            m14_t = cpool.tile([128, nb[14] * HS], f16, tag="m14")
            nc.sync.dma_start(out=m14_t[:], in_=m14_in[:])
            # m chunks: list of (tile, elem offset) per 16-block group
            m_chunks = [(m14_t, 512 * g) for g in range(ng[14])]

            for tr in reversed(transitions):   # d = 14 .. 1
                d = tr["d"]
                dd = d - 1                      # destination level
                entries = tr["entries"]
                empties = set(tr["empties"])
                ne = len(entries)
                nwd = nb[dd]                    # windows = blocks of dest level
                sel_tiles = []
                for c in range(0, ne, SELCH):
                    hi = min(c + SELCH, ne)
                    st = selpool.tile([128, SELCH * WS], f8, tag="sel")
                    nc.sync.dma_start(out=st[:, :(hi - c) * WS],
                                      in_=sel_in[d][:, c * WS:hi * WS])
                    sel_tiles.append(st)
                sc_t = pool.tile([128, nb[dd] * HS], f16, tag="scale")
                nc.sync.dma_start(out=sc_t[:], in_=scale_in[dd][:])

                new_chunks = []
                eidx = 0
                for g in range(ng[dd]):
                    wlo, whi = g * 16, min((g + 1) * 16, nwd)
                    nwin = whi - wlo
                    ps = psum_pool.tile([128, 512], f32, tag="ps")
                    covered = set()
                    while eidx < ne and entries[eidx][1] < whi:
                        s, t = entries[eidx]
                        first = t not in covered
                        covered.add(t)
                        last = (eidx + 1 == ne) or (entries[eidx + 1][1] != t)
                        st = sel_tiles[eidx // SELCH]
                        off = (eidx % SELCH) * WS
                        su = s
                        mt, mo = m_chunks[su // 16]
                        nc.tensor.matmul(
                            out=ps[:, (t % 16) * HS:(t % 16 + 1) * HS],
                            lhsT=st[:, off:off + WS],
                            rhs=mt[:, mo + (su % 16) * HS:mo + (su % 16 + 1) * HS],
                            start=first, stop=last)
                        eidx += 1
                    for t in range(wlo, whi):
                        if t in empties:
                            nc.vector.memset(ps[:, (t % 16) * HS:(t % 16 + 1) * HS], 0)
                    hp = pool.tile([128, nwin * HS], f32, tag=f"hp{g % 4}")
                    nc.vector.tensor_tensor(
                        out=hp[:], in0=ps[:, :nwin * HS],
                        in1=cb_t[:, :nwin * HS], op=mybir.AluOpType.add)
                    hh = pool.tile([128, nwin * HS], f16, tag=f"hh{g % 4}")
                    nc.scalar.activation(out=hh[:], in_=hp[:],
                                         func=mybir.ActivationFunctionType.Tanh)
                    mc = pool.tile([128, nwin * HS], f16, tag=f"mc{g % 8}")
                    nc.vector.tensor_tensor(
                        out=mc[:], in0=hh[:],
                        in1=sc_t[:, g * 512:g * 512 + nwin * HS],
                        op=mybir.AluOpType.mult)
                    new_chunks.append((mc, 0))
                m_chunks = new_chunks

            # ---- root reduce: sum all m_0 rows ----
            red_t = pool.tile([128, HS], f32, tag="red")
            for g, (mt, mo) in enumerate(m_chunks):
                nblk = min(16, nb[0] - g * 16)
                rc = pool.tile([128, HS], f32, tag="redc")
                ap = mt[:]
                nc.vector.tensor_reduce(
                    out=rc[:],
                    in_=bass.AP(ap.tensor, ap.offset + mo,
                                [[ap.ap[0][0], 128], [1, HS], [HS, nblk]]),
                    axis=mybir.AxisListType.X,
                    op=mybir.AluOpType.add)
                if g == 0:
                    nc.vector.tensor_copy(out=red_t[:], in_=rc[:])
                else:
                    nc.vector.tensor_tensor(out=red_t[:], in0=red_t[:],
                                            in1=rc[:], op=mybir.AluOpType.add)
            rps = psum_pool.tile([128, HS], f32, tag="ps")
            nc.tensor.matmul(out=rps[0:1, :], lhsT=ones_t[:], rhs=red_t[:],
                             start=True, stop=True)
            rout = pool.tile([1, HS], f32, tag="ro")
            nc.vector.tensor_copy(out=rout[:], in_=rps[0:1, :])
            nc.sync.dma_start(out=root_out[:], in_=rout[:])

    nc.finalize()
    return nc


def kernel(embedding, Wx, We, b, parent, etype, levels, is_rel):
    from concourse.bass_utils import run_bass_kernel_spmd

    embedding = np.asarray(embedding, np.float32)
    Wx = np.asarray(Wx, np.float32)
    We = np.asarray(We, np.float32)
    b = np.asarray(b, np.float32)
    parent = np.asarray(parent, np.int64)
    etype = np.asarray(etype, np.int64)
    levels_np = np.asarray(levels, np.int64)
    is_rel = np.asarray(is_rel, bool)

    key = (parent.tobytes(), is_rel.tobytes(), levels_np.tobytes())
    import hashlib
    key = hashlib.sha1(b"".join(key)).hexdigest()
    if key not in _cache:
        layouts, transitions = _build_structure(parent, levels_np, is_rel)
        nc = _compile(layouts, transitions)
        _cache[key] = (layouts, transitions, nc)
    layouts, transitions, nc = _cache[key]

    # ---- numeric inputs ----
    c = embedding @ Wx                       # [H]
    cb_full = c + b[0]                       # [H]
    tanhcb = np.tanh(cb_full)
    WeT = We[:, 0, :]                        # [E, H]

    nbs = [L["nslot"] // 128 for L in layouts]
    in_maps = []
    for core in range(NCORES):
        cs = slice(core * HS, (core + 1) * HS)
        cb_c = cb_full[cs]
        m = {"cb": np.tile(cb_c, (128, 16)).astype(np.float32),
             "ones": np.ones((128, 1), np.float32)}
        for tr in transitions:
            m[f"sel{tr['d']}"] = tr["sel"]
        # scale per level 0..13; m14 for level 14
        for d in range(D - 1):
            L = layouts[d]
            sn = L["slot_node"]
            nb = nbs[d]
            sc = np.zeros((L["nslot"], HS), np.float32)
            real = sn >= 0
            sc[real] = WeT[etype[sn[real]]][:, cs]
            scr = sc.reshape(nb, 128, HS).transpose(1, 0, 2).reshape(128, nb * HS)
            if d == D - 2:
                m["m14"] = (scr * np.tile(tanhcb[cs], nb)[None, :]).astype(np.float16)
            else:
                m[f"scale{d}"] = scr.astype(np.float16)
        in_maps.append(m)

    trace = bool(os.environ.get("CSRNN_TRACE"))
    kw = {}
    if trace:
        import tempfile
        _install_profhook()
        kw = {"trace": True, "tmpdir": tempfile.mkdtemp(prefix="csrnn_")}
    res = run_bass_kernel_spmd(nc, in_maps, list(range(NCORES)), **kw)
    global LAST_EXEC_NS
    LAST_EXEC_NS = res.exec_time_ns
    acc0 = np.concatenate([res.results[core]["root"][0] for core in range(NCORES)])
    root_hidden = np.zeros(H, np.float32) if is_rel[0] else acc0
    out = np.tanh(c + root_hidden + b[0])
    return out[None, :].astype(np.float32)
